# revision 73
# baseline (speedup 1.0000x reference)
"""Cross-attention kernel for Trainium2, distributed over 8 NeuronCores.

Problem: B=4, Sk=4096, Sq=2048, d_model=1024, dims=64 (fp32 reference).

Sharding (hardcoded): core c -> (batch b = c//2, K-half g = c%2).
Each core computes UN-NORMALIZED partial attention (numerator + softmax
denominator) of ALL 2048 decoder rows against its half of the encoder keys
(rows g*2048:(g+1)*2048). The host sums the two partials per batch and
normalizes (flash-attention style split-K; exp has no max-subtraction, so
partials combine by plain addition). No collectives.

Per-core dataflow (all bf16 operands; layouts avoid large on-chip
transposes):
  - Host pre-transposes/casts to bf16: encT [1024, 2048], decT [1024, 2048]
    (d_model on partitions).
  - KV^T projection: lhsT = [Wv | Wk] [128d, 128], rhs = encT chunks
    -> psum [128, 512]: rows 0:64 = V^T, 64:128 = K^T. Full PE array.
  - V^T is evacuated into a tile whose row 64 is constant 1.0; PE transposes
    yield V-natural blocks [128k, 65] whose col 64 is the ones column ->
    AV lhsT directly, so the ones column accumulates the softmax denominator
    during the AV matmul.
  - Scores computed transposed, ROW-TILED: kTd holds even k-blocks' K^T on
    partitions 0:64 and odd k-blocks' on 64:128; the two S matmuls of a
    k-block pair run in opposite PE row-halves (tile_position (0,0)/(64,0))
    so they execute concurrently on HW. qTd duplicated on both halves.
  - exp on ACT (exact, bf16 out); PSUM evacuations and copies on DVE.
  - AV: lhsT = vnat [128k, 65] (V block + ones col), rhs = at tiles; psum
    out^T [65, 512] per decoder-column half accumulated over k blocks.
  - The 2048 decoder rows are processed in TWO passes of 1024 columns
    (PSUM budget); K/V stay resident in SBUF across passes.
  - Output: un-normalized out^T [65, 512] per quarter straight to DRAM
    (after one PSUM->SBUF copy); host divides by the denominator row.
"""

import numpy as np
import ml_dtypes

import concourse.bass as bass
import concourse.bacc as bacc
import concourse.tile as tile
from concourse import mybir
from concourse._compat import with_exitstack
from concourse.bass_utils import run_bass_kernel_spmd
from concourse.masks import make_identity
from concourse.dve_spec import Spec, Src0, C0, C1, C2, Zero, lower, Bin, AluOp
from concourse.dve_uop import DveOpSpec
from concourse import dve_ops as _dve_ops

BF16 = mybir.dt.bfloat16
F32 = mybir.dt.float32
U16 = mybir.dt.uint16

B, SK, SQ_FULL, D, DIMS = 4, 4096, 2048, 1024, 64
N_CORES = 8
SKC = SK // 2      # 2048 encoder rows per core (K-half)
SQC = SQ_FULL      # 2048 decoder rows per core (all of them)
DC = D // 128      # d_model chunks of 128
KBLKS = SKC // 128  # 16 k blocks
NCK = SKC // 512   # 4 kv chunks
NJ = SQC // 512    # 4 decoder column halves
NPASS = 2          # process decoder halves in two passes of 1024

# exp engine split by query column; ACT owns everything by default (DVE's
# Schraudolph path costs ~3x per column in overheads and ACT has headroom).
QA = 512
QD = 512 - QA

SCHRAU_A = 128.0 * float(np.log2(np.e))
SCHRAU_B = 127.0 * 128.0
MAGIC = float(1.5 * 2.0**30)
CORR_A = 0.3436341389096249


def _exp2fix_op():
    for op in _dve_ops.OPS:
        if op.name == "EXP2FIX_ANT":
            return op
    p = Src0 + C0
    r = p - C0
    d = Src0 - r
    g = Bin(AluOp.ABSOLUTE_DIFF, d, Zero)
    gm = g + C1
    body = Src0 + (g * gm) * C2
    spec = Spec(body=body)
    shas = {
        ver: DveOpSpec(name="EXP2FIX_ANT", uops=lower(spec, ver=ver)).sha(ver)
        for ver in ("v3", "v4")
    }
    op = _dve_ops.DveOp("EXP2FIX_ANT", spec, subdim=False, uops_sha=shas)
    _dve_ops.OPS.append(op)
    _dve_ops.CUSTOM_DVE_SPECS[op.name] = spec
    _dve_ops._SUB_OPCODE_FOR_NAME[op.name] = (
        _dve_ops._CUSTOM_DVE_ROW_BASE + len(_dve_ops.OPS) - 1
    )
    return op


EXP2FIX = _exp2fix_op()


@with_exitstack
def _body(ctx, tc, encT, decT, wkv, wq, bv, bk, bq, out):
    nc = tc.nc

    singles = ctx.enter_context(tc.tile_pool(name="singles", bufs=1))
    loads = ctx.enter_context(tc.tile_pool(name="loads", bufs=1))
    ps_pool = ctx.enter_context(tc.tile_pool(name="ps", bufs=2, space="PSUM"))
    po_pool = ctx.enter_context(tc.tile_pool(name="po", bufs=2, space="PSUM"))
    at_pool = ctx.enter_context(tc.tile_pool(name="at", bufs=6))

    wkv_sb = singles.tile([128, DC, 128], BF16)
    wq_sb = singles.tile([128, DC, 128], BF16)
    bv_sb = singles.tile([DIMS, 1], F32)
    bk_sb = singles.tile([128, 1], F32)
    bq_sb = singles.tile([128, 1], F32)
    ident_bf = singles.tile([128, 128], BF16)
    make_identity(nc, ident_bf)

    # warm-up: keep PE continuously busy while the first DMAs stream so the
    # clock ramp (HAM) completes before the first real matmul
    for w in range(40):
        pw = ps_pool.tile([128, 128], BF16, tag="aux", name=f"warm{w % 2}")
        nc.tensor.transpose(pw, ident_bf, ident_bf)

    # persistent activations (resident across both decoder passes)
    kTd = singles.tile([128, SKC // 2], BF16)
    vTx = singles.tile([80, SKC], BF16)
    nc.vector.memset(vTx[DIMS:DIMS + 1, :], 1.0)
    vnat = singles.tile([128, KBLKS, 80], BF16)
    qTd = singles.tile([128, SQC], BF16)
    oT = singles.tile([DIMS + 1, 1024], F32)

    # activation loads: enc pairs (two 4-d-chunk tiles per 1024 cols), dec
    esbs = []
    for kp in range(SKC // 1024):
        e0 = loads.tile([128, 4, 1024], BF16, tag=f"esb{kp}a", name=f"esb{kp}a")
        e1 = loads.tile([128, 4, 1024], BF16, tag=f"esb{kp}b", name=f"esb{kp}b")
        esbs.append((e0, e1))
    dsb = loads.tile([128, DC, SQC], BF16, tag="dload")

    enc_r = encT.rearrange("(c p) n -> p c n", p=128)
    dec_r = decT.rearrange("(c p) n -> p c n", p=128)

    # DMA issue order: first decoder half-column block (d-split) and first
    # enc columns (256-col pieces) interleaved with the weights; pass-1
    # decoder cols last.
    nc.sync.dma_start(out=dsb[:, 0:4, 0:512], in_=dec_r[:, 0:4, 0:512])
    nc.sync.dma_start(out=wq_sb, in_=wq)
    nc.sync.dma_start(out=bq_sb, in_=bq)
    nc.sync.dma_start(out=dsb[:, 4:8, 0:512], in_=dec_r[:, 4:8, 0:512])
    nc.sync.dma_start(out=wkv_sb, in_=wkv)
    nc.sync.dma_start(out=esbs[0][0][:, :, 0:256], in_=enc_r[:, 0:4, 0:256])
    nc.sync.dma_start(out=esbs[0][1][:, :, 0:256], in_=enc_r[:, 4:8, 0:256])
    nc.sync.dma_start(out=bv_sb, in_=bv)
    nc.sync.dma_start(out=bk_sb, in_=bk)
    nc.sync.dma_start(out=esbs[0][0][:, :, 256:512], in_=enc_r[:, 0:4, 256:512])
    nc.sync.dma_start(out=esbs[0][1][:, :, 256:512], in_=enc_r[:, 4:8, 256:512])
    nc.sync.dma_start(out=dsb[:, 0:4, 512:1024], in_=dec_r[:, 0:4, 512:1024])
    nc.sync.dma_start(out=dsb[:, 4:8, 512:1024], in_=dec_r[:, 4:8, 512:1024])
    for h in range(2):
        nc.sync.dma_start(
            out=esbs[0][h][:, :, 512:1024], in_=enc_r[:, 4 * h:4 * h + 4, 512:1024]
        )
    for half in range(2):
        sl = slice(1024 + half * 512, 1024 + half * 512 + 512)
        for h in range(2):
            nc.sync.dma_start(
                out=esbs[1][h][:, :, half * 512:half * 512 + 512],
                in_=enc_r[:, 4 * h:4 * h + 4, sl],
            )
    nc.sync.dma_start(out=dsb[:, :, 1024:2048], in_=dec_r[:, :, 1024:2048])

    # --- K/V projection / evacuation / V transpose (once, pass-independent)
    def kv_mms(ck):
        pskv = ps_pool.tile([128, 512], F32, tag="aux", name=f"pskv{ck % 2}")
        for d in range(DC):
            esb = esbs[ck // 2][d // 4]
            nc.tensor.matmul(
                pskv, lhsT=wkv_sb[:, d, :],
                rhs=esb[:, d % 4, (ck % 2) * 512:(ck % 2 + 1) * 512],
                start=(d == 0), stop=(d == DC - 1),
            )
        return pskv

    def kv_evac(ck, pskv):
        sl = slice(ck * 512, (ck + 1) * 512)
        nc.vector.tensor_scalar_add(vTx[0:DIMS, sl], pskv[0:DIMS, :], bv_sb)
        pk = pskv[DIMS:128, :].rearrange("p (s n) -> p s n", n=128)
        kt = kTd[:, ck * 256:(ck + 1) * 256].rearrange("p (s n) -> p s n", n=128)
        nc.vector.tensor_scalar_add(kt[0:64, :, :], pk[:, 0::2, :], bk_sb[0:64, :])
        nc.vector.tensor_scalar_add(kt[64:128, :, :], pk[:, 1::2, :], bk_sb[64:128, :])

    def v_trans_kbs(kbs):
        for kb in kbs:
            ptv = ps_pool.tile([128, DIMS + 1], BF16, tag="aux", name=f"ptv{kb % 2}")
            nc.tensor.transpose(
                ptv, vTx[0:DIMS + 1, kb * 128:(kb + 1) * 128],
                ident_bf[0:DIMS + 1, 0:DIMS + 1],
            )
            nc.vector.tensor_copy(vnat[:, kb, 0:DIMS + 1], ptv)

    def v_trans(ck):
        v_trans_kbs(range(ck * 4, (ck + 1) * 4))

    # 256-col sub-half of kv chunk ck (= one k-block pair g = 2*ck + sub);
    # used for chunk 0 so the first S matmuls start as soon as possible
    def kv_mms_q(ck, sub):
        pskv = ps_pool.tile([128, 256], F32, tag="aux", name=f"pskvq{sub}")
        base = (ck % 2) * 512 + sub * 256
        for d in range(DC):
            esb = esbs[ck // 2][d // 4]
            nc.tensor.matmul(
                pskv, lhsT=wkv_sb[:, d, :], rhs=esb[:, d % 4, base:base + 256],
                start=(d == 0), stop=(d == DC - 1),
            )
        return pskv

    def kv_evac_q(ck, sub, pskv):
        g = 2 * ck + sub
        sl = slice(ck * 512 + sub * 256, ck * 512 + sub * 256 + 256)
        nc.vector.tensor_scalar_add(vTx[0:DIMS, sl], pskv[0:DIMS, :], bv_sb)
        nc.vector.tensor_scalar_add(
            kTd[0:64, g * 128:(g + 1) * 128], pskv[DIMS:128, 0:128], bk_sb[0:64, :]
        )
        nc.vector.tensor_scalar_add(
            kTd[64:128, g * 128:(g + 1) * 128], pskv[DIMS:128, 128:256],
            bk_sb[64:128, :],
        )

    # --- Q projection for one 512-col decoder half ---
    def q_proj(jq):
        psq = ps_pool.tile([128, 512], F32, tag="aux", name=f"psq{jq}")
        for d in range(DC):
            nc.tensor.matmul(
                psq, lhsT=wq_sb[:, d, :], rhs=dsb[:, d, jq * 512:(jq + 1) * 512],
                start=(d == 0), stop=(d == DC - 1),
            )
        nc.vector.tensor_scalar_add(qTd[:, jq * 512:(jq + 1) * 512], psq, bq_sb)

    # --- S (row-tiled pair) + exp for pair g against pass p's two q halves
    at_tiles = {}

    def s_pair_j(g, psses, p, jj):
        jq = 2 * p + jj
        for half in (0, 1):
            nc.tensor.matmul(
                psses[half][:, jj, :],
                lhsT=kTd[64 * half:64 * half + 64, g * 128:(g + 1) * 128],
                rhs=qTd[64 * half:64 * half + 64, jq * 512:(jq + 1) * 512],
                start=True, stop=True,
            )

    def s_pair(g, p):
        psses = [
            ps_pool.tile([128, 2, 512], F32, tag="ps", name=f"pss{h}")
            for h in (0, 1)
        ]
        for jj in (0, 1):
            s_pair_j(g, psses, p, jj)
        return psses

    def exp_pair(g, p, psses, jj_split=False, qd=0):
        # qd > 0 offloads columns [512-qd, 512) to DVE's corrected-Schraudolph
        # exp (unbiased, so mixing with exact exp per-column is safe); used
        # where ACT is locally the bottleneck (pass transition, tail).
        qa = 512 - qd
        for i, kb in enumerate((2 * g, 2 * g + 1)):
            at = at_pool.tile([128, 2, 512], BF16, tag="at", name=f"at{kb % 4}")
            at_tiles[kb] = at
        if jj_split:
            # finer exp grain for the last chunk, jj-major so the j=0 AV
            # chain (and its output stage) completes after two exps
            for jj in (0, 1):
                for i, kb in enumerate((2 * g, 2 * g + 1)):
                    nc.scalar.activation(
                        at_tiles[kb][:, jj, 0:qa], psses[i][:, jj, 0:qa],
                        mybir.ActivationFunctionType.Exp,
                    )
        else:
            for i, kb in enumerate((2 * g, 2 * g + 1)):
                nc.scalar.activation(
                    at_tiles[kb][:, :, 0:qa], psses[i][:, :, 0:qa],
                    mybir.ActivationFunctionType.Exp,
                )

    # ---------------- schedule ----------------
    # prologue: Q proj for pass-0 halves, kv chunk 0 in 256-col halves (so
    # the first S matmuls and exps start as early as possible), kv chunk 1
    q_proj(0)
    pa = kv_mms_q(0, 0)
    kv_evac_q(0, 0, pa)
    v_trans_kbs((0, 1))
    psses0 = [
        ps_pool.tile([128, 2, 512], F32, tag="ps", name=f"pss{h}") for h in (0, 1)
    ]
    s_pair_j(0, psses0, 0, 0)
    for i, kb in ((0, 0), (1, 1)):
        at = at_pool.tile([128, 2, 512], BF16, tag="at", name=f"at{kb % 4}")
        at_tiles[kb] = at
        nc.scalar.activation(
            at[:, 0, 0:QA], psses0[i][:, 0, 0:QA],
            mybir.ActivationFunctionType.Exp,
        )
    q_proj(1)
    pb = kv_mms_q(0, 1)
    kv_evac_q(0, 1, pb)
    v_trans_kbs((2, 3))
    s_pair_j(0, psses0, 0, 1)
    for i, kb in ((0, 0), (1, 1)):
        nc.scalar.activation(
            at_tiles[kb][:, 1, 0:QA], psses0[i][:, 1, 0:QA],
            mybir.ActivationFunctionType.Exp,
        )
    pskv1 = kv_mms(1)
    kv_evac(1, pskv1)
    psses1 = s_pair(1, 0)
    exp_pair(1, 0, psses1)
    v_trans(1)

    bridge = []
    for p in range(NPASS):
        po0 = po_pool.tile([DIMS + 1, 512], F32, tag="po", name=f"po0_{p}")
        po1 = po_pool.tile([DIMS + 1, 512], F32, tag="po", name=f"po1_{p}")
        pos = [po0, po1]

        def av_kb(kb, jj):
            nc.tensor.matmul(
                pos[jj], lhsT=vnat[:, kb, 0:DIMS + 1], rhs=at_tiles[kb][:, jj, :],
                start=(kb == 0), stop=(kb == KBLKS - 1),
            )

        def out_stage(jj):
            jq = 2 * p + jj
            nc.vector.tensor_copy(oT[:, jj * 512:(jj + 1) * 512], pos[jj])
            nc.sync.dma_start(out=out[jq], in_=oT[:, jj * 512:(jj + 1) * 512])

        for ck in range(1, NCK):
            for g in (2 * ck, 2 * ck + 1):
                psses = s_pair(g, p)
                exp_pair(g, p, psses, jj_split=(ck == NCK - 1 and g == 2 * ck + 1))
            if p == 0 and ck + 1 < NCK:
                pskv = kv_mms(ck + 1)
                kv_evac(ck + 1, pskv)
            if p == 0 and ck == NCK - 1:
                # pass-1 Q projections; their evacuation overlaps the AVs
                q_proj(2)
                q_proj(3)
            for kb in range(4 * (ck - 1), 4 * ck):
                for jj in (0, 1):
                    av_kb(kb, jj)
            if p == 0 and ck + 1 < NCK:
                v_trans(ck + 1)
        if p == 0:
            # bridge: pass-1 chunk-0 scores+exp emitted BEFORE the final AV
            # batch -- ACT is gapless through the whole back half, so getting
            # its pass-1 queue started earlier shifts the entire tail left
            for g in (0, 1):
                psses = s_pair(g, 1)
                exp_pair(g, 1, psses)
            bridge.append(True)
        for jj in (0, 1):
            for kb in range(KBLKS - 4, KBLKS):
                av_kb(kb, jj)
            out_stage(jj)


_NC_CACHE = None


def _build():
    global _NC_CACHE
    if _NC_CACHE is not None:
        return _NC_CACHE
    nc = bacc.Bacc(
        "TRN2", target_bir_lowering=False, debug=False,
        enable_asserts=True, num_devices=N_CORES,
    )
    encT = nc.dram_tensor("encT", [D, SKC], BF16, kind="ExternalInput").ap()
    decT = nc.dram_tensor("decT", [D, SQC], BF16, kind="ExternalInput").ap()
    # weights pre-laid-out host-side as [partition, d-chunk, col] so the DMA
    # reads 2KB contiguous per partition
    wkv = nc.dram_tensor("wkv", [128, DC, 128], BF16, kind="ExternalInput").ap()
    wq = nc.dram_tensor("wq", [128, DC, 128], BF16, kind="ExternalInput").ap()
    bv = nc.dram_tensor("bv", [DIMS, 1], F32, kind="ExternalInput").ap()
    bk = nc.dram_tensor("bk", [128, 1], F32, kind="ExternalInput").ap()
    bq = nc.dram_tensor("bq", [128, 1], F32, kind="ExternalInput").ap()
    # un-normalized partial out^T per decoder column half; row 64 = partial
    # softmax denominator
    out = nc.dram_tensor("out", [NJ, DIMS + 1, 512], F32, kind="ExternalOutput").ap()
    with tile.TileContext(nc) as tc:
        _body(tc, encT, decT, wkv, wq, bv, bk, bq, out)
    nc.compile()
    _NC_CACHE = nc
    return nc


def make_in_maps(**inputs):
    bf16 = ml_dtypes.bfloat16
    enc = np.asarray(inputs["encoder_output"])
    dec = np.asarray(inputs["decoder"])
    scale = DIMS ** -0.5

    def pcm(w):  # [1024, 128] -> [128 partition, 8 chunk, 128 col]
        return np.ascontiguousarray(
            w.reshape(DC, 128, 128).transpose(1, 0, 2)
        ).astype(bf16)

    wq1 = np.asarray(inputs["Wq"]) * scale
    wq_s = pcm(np.concatenate([wq1, wq1], axis=1))
    bq1 = (np.asarray(inputs["bq"]) * scale).astype(np.float32).reshape(DIMS, 1)
    bq_s = np.concatenate([bq1, bq1], axis=0)
    wkv = pcm(np.concatenate(
        [np.asarray(inputs["Wv"]), np.asarray(inputs["Wk"])], axis=1
    ))
    bv = np.asarray(inputs["bv"]).astype(np.float32).reshape(DIMS, 1)
    bk1 = np.asarray(inputs["bk"]).astype(np.float32).reshape(DIMS, 1)
    bk_s = np.concatenate([bk1, bk1], axis=0)
    in_maps = []
    for c in range(N_CORES):
        b, g = divmod(c, 2)
        in_maps.append({
            "encT": enc[b, g * SKC:(g + 1) * SKC, :].T.astype(bf16),
            "decT": dec[b].T.astype(bf16),
            "wkv": wkv, "wq": wq_s, "bv": bv, "bk": bk_s, "bq": bq_s,
        })
    return in_maps


def assemble(results):
    out = np.zeros((B, SQ_FULL, DIMS), np.float32)
    for b in range(B):
        oT = results[2 * b]["out"] + results[2 * b + 1]["out"]  # [NJ, 65, 512]
        o = (oT[:, 0:DIMS, :] / oT[:, DIMS:DIMS + 1, :]).transpose(0, 2, 1)
        out[b] = o.reshape(SQ_FULL, DIMS)
    return out


def kernel(**inputs) -> np.ndarray:
    nc = _build()
    in_maps = make_in_maps(**inputs)
    res = run_bass_kernel_spmd(nc, in_maps, core_ids=list(range(N_CORES)))
    return assemble(res.results)
